# revision 29
# baseline (speedup 1.0000x reference)
"""Trainium2 Bass kernel for nn_CrossLayer: out = LayerNorm(x0 * (x1@w) + x0).

Math: s = x1 @ w (per-row scalar), y = x0*s + x0, out = LN(y)*gamma + beta.
Per 128-row tile (H=2048 free dim):
    DVE : scalar_tensor_tensor + accum -> s = rowsum((x1*1)*w_b), in 16
          column chunks of 128 so the sequential-sum error stays at the
          reference matmul's own fp32 noise (out is a stride-0 dummy so the
          product never lands in SBUF)
    DVE : scalar_tensor_tensor + accum -> y = (x0*s)+x0, ysum     (1 pass;
          bit-identical rounding to the reference's x0*s + x0)
    ACT : activation(Square, bias=-mean, accum) -> ss = sum((y-mean)^2)
    tiny: rstd = 1/sqrt(ss/H + eps) via ACT Sqrt + DVE reciprocal + 2 Newton
          steps (ACT Sqrt spline alone is low-precision); b = -mean*rstd
    ACT : activation(Identity, scale=rstd, bias=b) -> out = y*rstd + b
Schedule (TimelineSim: 144.0us/core vs 139.8us mandatory DMA busy -- the
48 1MB tile transfers run back-to-back with zero gaps; HBM-bound as
targeted, 48MB/core at ~344GB/s):
  - input loads on the SP HWDGE ring, output stores on the ACT ring --
    separate rings avoid head-of-line blocking of stores behind future loads
  - stores split in 2 column chunks so DMA starts after the first apply half
  - w broadcast built on-chip (8KB row load + PE rank-1 matmul + PSUM copy)
    instead of a 1MB HBM broadcast DMA at the stream head
  - last tile's x1 load + s-chain hoisted to the kernel head, and the last 3
    tiles' apply on DVE (2x tensor_scalar mode) to shorten the drain tail
Sharding: pure data parallel, rows split across 8 cores; weight/gamma/beta
replicated. gamma==1/beta==0 detected host-side and folded away (the general
affine path applies two extra vector passes).
"""

import numpy as np

B, H = 16384, 2048
N_CORES = 8
ROWS = B // N_CORES          # rows per core
P = 128                      # partitions
NT = ROWS // P               # tiles per core
SCH = 16                     # s-reduction chunks
SCW = H // SCH               # chunk width (128)
LN_EPS = 1e-12

_cache = {}

IO_BUFS = 4
WORK_BUFS = 2
SMALL_BUFS = 4
APPLY_ON = "act"         # final normalize pass engine: "act" | "dve"
APPLY_DVE_TAIL = 3       # run apply on DVE for the last N tiles (frees ACT in the tail)
W_BCAST = "pe"           # build w broadcast via PE rank-1 matmul ("pe") or HBM DMA ("dma")
PREFETCH_N = 1           # hoist last tile x1 load + s-chain to kernel head
SPLIT_OUT = 2            # split final apply+store into N column chunks
SPLIT_IN = 1             # split input loads into N column chunks (1 = full-tile DMAs)
OUT_DMA_ENGINE = "act"   # ACT ring for stores: avoids HOL-blocking behind future input loads in the SP ring
X0_DMA_ENGINE = "sync"   # engine issuing x0 loads
X1_DMA_ENGINE = "sync"   # engine issuing x1 loads


def _build(apply_affine: bool):
    import concourse.bass as bass
    import concourse.bacc as bacc
    import concourse.tile as tile
    from concourse import mybir

    f32 = mybir.dt.float32
    op = mybir.AluOpType
    act_fn = mybir.ActivationFunctionType

    nc = bacc.Bacc("TRN2", target_bir_lowering=False, debug=False)
    x0 = nc.dram_tensor("x0", [ROWS, H], f32, kind="ExternalInput")
    x1 = nc.dram_tensor("x1", [ROWS, H], f32, kind="ExternalInput")
    w = nc.dram_tensor("weight", [H, 1], f32, kind="ExternalInput")
    if apply_affine:
        gamma = nc.dram_tensor("ln_gamma", [H], f32, kind="ExternalInput")
        beta = nc.dram_tensor("ln_beta", [H], f32, kind="ExternalInput")
    out = nc.dram_tensor("out", [ROWS, H], f32, kind="ExternalOutput")

    def bcast_rows(ap_1d):
        # [H] DRAM vector -> [P, H] SBUF tile via partition-stride-0 DMA
        return bass.AP(
            tensor=ap_1d.tensor,
            offset=ap_1d.offset,
            ap=[[0, P]] + list(ap_1d.ap),
        )

    with tile.TileContext(nc) as tc:
        with (
            tc.tile_pool(name="singles", bufs=1) as singles,
            tc.tile_pool(name="io", bufs=IO_BUFS) as io,
            tc.tile_pool(name="work", bufs=WORK_BUFS) as work,
            tc.tile_pool(name="small", bufs=SMALL_BUFS) as small,
        ):
            w_b = singles.tile([P, H], f32)
            if W_BCAST == "pe":
                # Broadcast w across partitions on-chip: load one 8KB row,
                # rank-1 matmul ones[1,P].T @ w_row[1,:] into PSUM, copy to
                # SBUF. Avoids a 1MB HBM broadcast DMA at the stream head.
                w_row = singles.tile([1, H], f32)
                w_ap = w[:, 0]
                nc.sync.dma_start(
                    out=w_row,
                    in_=bass.AP(
                        tensor=w_ap.tensor, offset=w_ap.offset,
                        ap=[[0, 1]] + list(w_ap.ap),
                    ),
                )
                ones_t = singles.tile([1, P], f32)
                nc.vector.memset(ones_t, 1.0)
                with tc.tile_pool(name="psum", bufs=1, space="PSUM") as psum:
                    w_ps = psum.tile([P, H], f32)
                    for j in range(H // 512):
                        nc.tensor.matmul(
                            out=w_ps[:, j * 512 : (j + 1) * 512],
                            lhsT=ones_t,
                            rhs=w_row[:, j * 512 : (j + 1) * 512],
                            start=True,
                            stop=True,
                        )
                    nc.scalar.copy(out=w_b, in_=w_ps)
            else:
                nc.sync.dma_start(out=w_b, in_=bcast_rows(w[:, 0]))
            if apply_affine:
                gamma_b = singles.tile([P, H], f32)
                nc.sync.dma_start(out=gamma_b, in_=bcast_rows(gamma[:]))
                beta_b = singles.tile([P, H], f32)
                nc.sync.dma_start(out=beta_b, in_=bcast_rows(beta[:]))
            eps_t = singles.tile([P, 1], f32)
            nc.vector.memset(eps_t, LN_EPS)
            dummy = singles.tile([P, 1], f32)

            def s_chain(x1_t, s_part, s):
                for j in range(SCH):
                    nc.vector.scalar_tensor_tensor(
                        out=dummy.broadcast_to([P, SCW]),
                        in0=x1_t[:, j * SCW : (j + 1) * SCW],
                        scalar=1.0,
                        in1=w_b[:, j * SCW : (j + 1) * SCW],
                        op0=op.mult,
                        op1=op.mult,
                        accum_out=s_part[:, j : j + 1],
                    )
                nc.vector.tensor_reduce(
                    out=s, in_=s_part, axis=mybir.AxisListType.X, op=op.add
                )

            # Hoist the last N tiles' x1 loads + s computation to the head so
            # the kernel tail (after the final input DMA) is just
            # y->stats->apply for those tiles.
            s_pre = {}
            for i in range(NT - PREFETCH_N, NT):
                rL = i * P
                x1_pre = singles.tile([P, H], f32, name=f"x1_pre{i}")
                nc.sync.dma_start(out=x1_pre, in_=x1[rL : rL + P, :])
                sp_pre = singles.tile([P, SCH], f32, name=f"sp_pre{i}")
                s_pre[i] = singles.tile([P, 1], f32, name=f"s_pre{i}")
                s_chain(x1_pre, sp_pre, s_pre[i])

            for i in range(NT):
                r0 = i * P
                last = i in s_pre
                CI = H // SPLIT_IN
                x0_eng = nc.scalar if X0_DMA_ENGINE == "act" else nc.sync
                x1_eng = nc.scalar if X1_DMA_ENGINE == "act" else nc.sync
                x0_t = io.tile([P, H], f32, tag="x0")
                for j in range(SPLIT_IN):
                    sl = slice(j * CI, (j + 1) * CI)
                    x0_eng.dma_start(out=x0_t[:, sl], in_=x0[r0 : r0 + P, sl])
                if last:
                    s = s_pre[i]
                else:
                    x1_t = io.tile([P, H], f32, tag="x1")
                    for j in range(SPLIT_IN):
                        sl = slice(j * CI, (j + 1) * CI)
                        x1_eng.dma_start(out=x1_t[:, sl], in_=x1[r0 : r0 + P, sl])
                    # s = rowsum(x1 * w), chunked to bound sequential-sum
                    # error near the reference matmul's own fp32 noise (16
                    # chunks of 128 + small combine ~= PE K-tiling). STT out
                    # is a stride-0 dummy (never read).
                    s_part = small.tile([P, SCH], f32, tag="s_part")
                    s = small.tile([P, 1], f32, tag="s")
                    s_chain(x1_t, s_part, s)

                # y = (x0 * s) + x0 (same rounding as reference), ysum for mean
                y_t = io.tile([P, H], f32, tag="y")
                ysum = small.tile([P, 1], f32, tag="ysum")
                nc.vector.scalar_tensor_tensor(
                    out=y_t,
                    in0=x0_t,
                    scalar=s,
                    in1=x0_t,
                    op0=op.mult,
                    op1=op.add,
                    accum_out=ysum,
                )

                # negm = -mean(y)
                negm = small.tile([P, 1], f32, tag="negm")
                nc.vector.tensor_scalar_mul(out=negm, in0=ysum, scalar1=-1.0 / H)

                # ss = sum((y - mean)^2); squares go to a junk tile
                junk = work.tile([P, H], f32, tag="junk")
                ss = small.tile([P, 1], f32, tag="ss")
                nc.scalar.activation(
                    out=junk,
                    in_=y_t,
                    func=act_fn.Square,
                    bias=negm,
                    scale=1.0,
                    accum_out=ss,
                )

                # q = ss/H + eps; rstd = 1/sqrt(q) with 2 Newton refinements
                # (ACT Sqrt spline is low-precision; NR restores ~1 ulp)
                q = small.tile([P, 1], f32, tag="q")
                nc.vector.tensor_scalar(
                    out=q, in0=ss, scalar1=1.0 / H, scalar2=LN_EPS,
                    op0=op.mult, op1=op.add,
                )
                t = small.tile([P, 1], f32, tag="t")
                nc.scalar.activation(out=t, in_=q, func=act_fn.Sqrt)
                r = small.tile([P, 1], f32, tag="r")
                nc.vector.reciprocal(out=r, in_=t)
                u = small.tile([P, 1], f32, tag="u")
                for _ in range(2):
                    nc.vector.tensor_mul(out=u, in0=r, in1=r)
                    nc.vector.tensor_mul(out=u, in0=u, in1=q)
                    nc.vector.tensor_scalar(
                        out=u, in0=u, scalar1=-0.5, scalar2=1.5,
                        op0=op.mult, op1=op.add,
                    )
                    nc.vector.tensor_mul(out=r, in0=r, in1=u)
                b_sc = small.tile([P, 1], f32, tag="b")
                nc.vector.tensor_mul(out=b_sc, in0=negm, in1=r)

                # out = y*rstd + b  (== (y-mean)*rstd)
                out_t = io.tile([P, H], f32, tag="out")
                CW = H // SPLIT_OUT
                for j in range(SPLIT_OUT):
                    sl = slice(j * CW, (j + 1) * CW)
                    if APPLY_ON == "dve" or i >= NT - APPLY_DVE_TAIL:
                        nc.vector.tensor_scalar(
                            out=out_t[:, sl], in0=y_t[:, sl], scalar1=r,
                            scalar2=b_sc, op0=op.mult, op1=op.add,
                        )
                    else:
                        nc.scalar.activation(
                            out=out_t[:, sl], in_=y_t[:, sl],
                            func=act_fn.Identity, bias=b_sc, scale=r,
                        )

                if apply_affine:
                    nc.vector.scalar_tensor_tensor(
                        out=out_t,
                        in0=out_t,
                        scalar=0.0,
                        in1=gamma_b,
                        op0=op.add,
                        op1=op.mult,
                    )
                    nc.vector.tensor_add(out=out_t, in0=out_t, in1=beta_b)

                out_eng = nc.scalar if OUT_DMA_ENGINE == "act" else nc.sync
                for j in range(SPLIT_OUT):
                    sl = slice(j * (H // SPLIT_OUT), (j + 1) * (H // SPLIT_OUT))
                    out_eng.dma_start(out=out[r0 : r0 + P, sl], in_=out_t[:, sl])

    nc.compile()
    return nc


LAST_RESULTS = None


def kernel(x0, x1, weight, ln_gamma, ln_beta):
    from concourse.bass_utils import run_bass_kernel_spmd

    global LAST_RESULTS
    x0 = np.asarray(x0, dtype=np.float32)
    x1 = np.asarray(x1, dtype=np.float32)
    weight = np.asarray(weight, dtype=np.float32)
    ln_gamma = np.asarray(ln_gamma, dtype=np.float32)
    ln_beta = np.asarray(ln_beta, dtype=np.float32)

    apply_affine = not (
        np.all(ln_gamma == 1.0) and np.all(ln_beta == 0.0)
    )
    if apply_affine not in _cache:
        _cache[apply_affine] = _build(apply_affine)
    nc = _cache[apply_affine]

    in_maps = []
    for k in range(N_CORES):
        m = {
            "x0": x0[k * ROWS : (k + 1) * ROWS],
            "x1": x1[k * ROWS : (k + 1) * ROWS],
            "weight": weight,
        }
        if apply_affine:
            m["ln_gamma"] = ln_gamma
            m["ln_beta"] = ln_beta
        in_maps.append(m)

    res = run_bass_kernel_spmd(nc, in_maps, core_ids=list(range(N_CORES)))
    LAST_RESULTS = res
    out = np.concatenate([res.results[k]["out"] for k in range(N_CORES)], axis=0)
    return (x0, out)
